# revision 38
# baseline (speedup 1.0000x reference)
"""Causal self-attention on 8 trn2 NeuronCores — v2.

Sharding: core = 2*b + g  (b in 0..3 batches, g in 0..1 head-groups of 8
heads). Host passes x^T per batch (so no on-chip transposes); per core:
  qkv^T = Wslice^T @ x^T   (feature-major), emitted interleaved with
  attention so the scalar engine's exp stream starts early.
  Attention in scores^T layout [k, q], both heads of a 128-partition
  group processed together (s=0 on PE rows 0:63, s=1 on rows 64:127).
  The AV lhsT is [V | ones] (128 cols), so the softmax denominator lands
  replicated on output partitions 64:128 — no cross-partition broadcast.
  Normalization: reciprocal_approx_fast + one multiply per (head, 1024 q).
  partial out^T = yT @ Wp_slice -> [1024, 2048], DMA'd from PSUM.
Host gathers: out[b] = (partial[2b] + partial[2b+1]).T + b_proj.
"""

import numpy as np
import ml_dtypes

B, T, E, H = 4, 2048, 1024, 16
HD = E // H  # 64

_CACHE = {}


def _build(debug=False):
    from contextlib import ExitStack

    import concourse.bass as bass
    import concourse.mybir as mybir
    import concourse.tile as tile
    from concourse import bacc

    F32 = mybir.dt.float32
    BF16 = mybir.dt.bfloat16
    AF = mybir.ActivationFunctionType
    ALU = mybir.AluOpType

    nc = bacc.Bacc("TRN2", target_bir_lowering=False)
    xin = nc.dram_tensor("xin", [128, 8, T], BF16, kind="ExternalInput")
    wqkv = nc.dram_tensor("wqkv", [12, 128, 8, 128], BF16, kind="ExternalInput")
    bqkv = nc.dram_tensor("bqkv", [128, 12], F32, kind="ExternalInput")
    wp = nc.dram_tensor("wp", [128, 4, 1024], BF16, kind="ExternalInput")
    outT = nc.dram_tensor("outT", [E, T], BF16, kind="ExternalOutput")
    if debug:
        qkvT_dbg = nc.dram_tensor(
            "qkvT_dbg", [128, 12, T], BF16, kind="ExternalOutput"
        )
        yT_dbg = nc.dram_tensor("yT_dbg", [128, 4, T], BF16, kind="ExternalOutput")

    with tile.TileContext(nc) as tc, ExitStack() as ctx:
        const = ctx.enter_context(tc.tile_pool(name="const", bufs=1))
        # stacked 64x64 identities at partitions 0 and 64 (v-transpose lhsT
        # sits at partition base 0 or 64)
        id2f = const.tile([128, 64], F32, tag="id2f")
        nc.gpsimd.memset(id2f[:], 0.0)
        for off in (0, 64):
            nc.gpsimd.affine_select(
                out=id2f[:],
                in_=id2f[:],
                compare_op=ALU.not_equal,
                fill=1.0,
                base=-off,
                pattern=[[-1, 64]],
                channel_multiplier=1,
            )
        id2 = const.tile([128, 64], BF16, tag="id2")
        nc.vector.tensor_copy(id2[:], id2f[:])
        from concourse.masks import make_identity

        identf = const.tile([128, 128], F32, tag="identf")
        make_identity(nc, identf[:])
        identr = const.tile([128, 128], BF16, tag="identr")
        nc.vector.tensor_copy(identr[:], identf[:])
        biasT = const.tile([128, 12], F32, tag="biasT")
        nc.sync.dma_start(biasT[:], bqkv[:])
        wps = const.tile([128, 4, 1024], BF16, tag="wps")
        ones64 = const.tile([128, 16, 64], BF16, tag="ones64")
        nc.gpsimd.memset(ones64[:], 1.0)
        # masks[j][p, col] = 1 iff col >= 128*j + p (causal keep in [k,q] layout)
        masks = []
        with tc.tile_pool(name="mtmp", bufs=2) as mtmp:
            for j in range(4):
                mjf = mtmp.tile([128, 512], F32, tag="maskf", name=f"maskf{j}")
                nc.gpsimd.memset(mjf[:], 1.0)
                nc.gpsimd.affine_select(
                    out=mjf[:],
                    in_=mjf[:],
                    compare_op=ALU.is_ge,
                    fill=0.0,
                    base=-128 * j,
                    pattern=[[1, 512]],
                    channel_multiplier=-1,
                )
                mj = const.tile([128, 512], BF16, tag=f"mask{j}", name=f"mask{j}")
                nc.vector.tensor_copy(mj[:], mjf[:])
                masks.append(mj)

        xT_pool = ctx.enter_context(tc.tile_pool(name="xT", bufs=1))
        xT = xT_pool.tile([128, 8, T], BF16, tag="xT")

        qkvT_pool = ctx.enter_context(tc.tile_pool(name="qkvT", bufs=1))
        qkvT = qkvT_pool.tile([128, 12, T], BF16, tag="qkvT")
        yT_pool = ctx.enter_context(tc.tile_pool(name="yT", bufs=1))
        yT = yT_pool.tile([128, 4, T], BF16, tag="yT")
        # vaug[:, s, kb, 0:64] = V block (k rows on partitions), cols 64:128
        # stay at the initial memset value 1.0 (denominator ones)


        with (
            tc.tile_pool(name="wq", bufs=4) as wq_pool,
            tc.tile_pool(name="vaug", bufs=2) as vaug_pool,
            tc.tile_pool(name="Pp", bufs=3) as P_pool,
            tc.tile_pool(name="rec", bufs=2) as rec_pool,
            tc.tile_pool(name="psBs", bufs=2, space="PSUM") as psBs,
            tc.tile_pool(name="psBy", bufs=2, space="PSUM") as psBy,
        ):
            # P buffers are partially overwritten each use; the mask multiply
            # reads the full 512 chunk, so zero them once up front.
            for s in range(2):
                for i in range(3):
                    Pt0 = P_pool.tile([128, 1024], BF16, tag=f"P{s}", name=f"Pt0_{s}_{i}")
                    nc.gpsimd.memset(Pt0[:], 0.0)

            # DMA order matters: the first QKV matmul needs wqm[0] and
            # xT[k=0] only — put them first so the PE starts ~2us in, and
            # defer wps (phase C) to the end of the input stream.
            wqms = [
                wq_pool.tile([128, 8, 128], BF16, tag="wqm", name=f"wqm{m}")
                for m in range(12)
            ]
            nc.sync.dma_start(wqms[0][:], wqkv[0])
            nc.sync.dma_start(wqms[1][:], wqkv[1])
            for k in range(8):
                nc.sync.dma_start(xT[:, k, :], xin[:, k, :])
            for m in range(2, 12):
                nc.sync.dma_start(wqms[m][:], wqkv[m])
            nc.sync.dma_start(wps[:], wp[:])

            def emit_a(m, jj):
                pq = psBs.tile([128, 1024], F32, tag="s", name=f"pq{m}_{jj}")
                for j2 in range(2):
                    for k in range(8):
                        nc.tensor.matmul(
                            pq[:, j2 * 512 : (j2 + 1) * 512],
                            wqms[m][:, k, :],
                            xT[:, k, (jj * 2 + j2) * 512 : (jj * 2 + j2 + 1) * 512],
                            start=(k == 0),
                            stop=(k == 7),
                        )
                nc.vector.tensor_scalar_add(
                    qkvT[:, m, jj * 1024 : (jj + 1) * 1024],
                    pq[:],
                    biasT[:, m : m + 1],
                )

            # A(0) runs up front; A(p+1) units are interleaved into p's
            # attention loop below — the 128-contraction QKV matmuls keep
            # the PE's activity monitor warm (attention-only streams run at
            # half clock) and give the PE work during exp waits.
            for m in range(3):
                for jj in range(2):
                    emit_a(m, jj)

            for p in range(4):
                # ---- V transposes into vaug [k, V(64)|ones(64)] per kb.
                # One matmul per kb transposes BOTH heads (contraction over
                # all 128 dim-partitions, identity rhs).
                vaug = vaug_pool.tile([128, 2, 2048], BF16, tag="vaug")
                vaug_k = vaug[:, :, :].rearrange("p s (kb c) -> p s kb c", kb=16, c=128)
                for s in range(2):
                    nc.vector.tensor_copy(vaug_k[:, s, :, 64:128], ones64[:])
                for h in range(2):
                    pvt = psBs.tile([128, 1024], F32, tag="s")
                    for i in range(8):
                        kb = 8 * h + i
                        nc.tensor.matmul(
                            pvt[:, i * 128 : (i + 1) * 128],
                            qkvT[:, 3 * p + 2, kb * 128 : (kb + 1) * 128],
                            identr[:],
                            start=True,
                            stop=True,
                        )
                    pvt_k = pvt[:, :].rearrange("p (a c) -> p a c", a=8, c=128)
                    for s in range(2):
                        nc.vector.tensor_copy(
                            vaug_k[:, s, 8 * h : 8 * h + 8, 0:64],
                            pvt_k[:, :, 64 * s : 64 * s + 64],
                        )

                # ---- attention for heads (p, s=0), (p, s=1), s-fused ----
                a_units = (
                    [(m, jj) for m in range(3 * p + 3, 3 * p + 6) for jj in range(2)]
                    if p < 3
                    else []
                )
                ui = 0
                kbc = 0
                qT = [qkvT[64 * s : 64 * s + 64, 3 * p, :] for s in range(2)]
                kT = [qkvT[64 * s : 64 * s + 64, 3 * p + 1, :] for s in range(2)]
                for qc in range(2):
                    q0 = qc * 1024
                    kmax = (qc + 1) * 8
                    klast = [
                        min(kmax, qc * 8 + (ci + 1) * 4) - 1 for ci in range(2)
                    ]
                    ymm = [
                        psBy.tile([128, 1024], F32, tag="ymm", name=f"ymm{p}_{qc}_{s}")
                        for s in range(2)
                    ]
                    for kb in range(kmax):
                        w0 = max(0, kb * 128 - q0)  # exact causal offset
                        c0 = (w0 // 512) * 512  # 512-aligned chunk start
                        sp = [
                            psBs.tile([128, 1024], F32, tag="s", name=f"sp{p}_{qc}_{kb}_{s}")
                            for s in range(2)
                        ]
                        for j in range(c0, 1024, 512):
                            for s in range(2):
                                nc.tensor.matmul(
                                    sp[s][:, j : j + 512],
                                    kT[s][:, kb * 128 : (kb + 1) * 128],
                                    qT[s][:, q0 + j : q0 + j + 512],
                                    start=True,
                                    stop=True,
                                    tile_position=(64 * s, 0),
                                )
                        pts = []
                        for s in range(2):
                            Pt = P_pool.tile(
                                [128, 1024], BF16, tag=f"P{s}", name=f"Pt{p}_{qc}_{kb}_{s}"
                            )
                            nc.scalar.activation(
                                Pt[:, w0:1024], sp[s][:, w0:1024], AF.Exp, scale=0.125
                            )
                            if kb >= qc * 8:  # diagonal block: causal mask
                                nc.vector.tensor_tensor(
                                    out=Pt[:, c0 : c0 + 512],
                                    in0=Pt[:, c0 : c0 + 512],
                                    in1=masks[(w0 - c0) // 128][:],
                                    op=ALU.mult,
                                )
                            pts.append(Pt)
                        for j in range(c0, 1024, 512):
                            for s in range(2):
                                nc.tensor.matmul(
                                    ymm[s][:, j : j + 512],
                                    vaug[:, s, kb * 128 : (kb + 1) * 128],
                                    pts[s][:, j : j + 512],
                                    start=(kb == 0),
                                    stop=(kb == klast[j // 512]),
                                )
                        kbc += 1
                        if kbc % 4 == 0 and ui < len(a_units):
                            emit_a(*a_units[ui])
                            ui += 1
                    for s in range(2):
                        den = rec_pool.tile([64, 1024], F32, tag="den")
                        nc.vector.tensor_copy(den[:], ymm[s][64:128, :])
                        rec = rec_pool.tile([64, 1024], F32, tag="rec")
                        nc.vector.reciprocal_approx_fast(rec[:], den[:])
                        nc.vector.tensor_tensor(
                            out=yT[64 * s : 64 * s + 64, p, q0 : q0 + 1024],
                            in0=ymm[s][0:64, :],
                            in1=rec[:],
                            op=ALU.mult,
                        )
                while ui < len(a_units):
                    emit_a(*a_units[ui])
                    ui += 1

        if debug:
            nc.sync.dma_start(qkvT_dbg[:], qkvT[:])
            nc.sync.dma_start(yT_dbg[:], yT[:])

        with (
            tc.tile_pool(name="ob", bufs=1) as ob_pool,
            tc.tile_pool(name="psC", bufs=8, space="PSUM") as psC,
        ):
            for m in range(8):
                pn = [
                    psC.tile([128, 512], F32, tag="pc", name=f"pc{m}_{n}")
                    for n in range(4)
                ]
                for k in range(4):
                    for n in range(4):
                        nc.tensor.matmul(
                            pn[n][:],
                            wps[:, k, m * 128 : (m + 1) * 128],
                            yT[:, k, n * 512 : (n + 1) * 512],
                            start=(k == 0),
                            stop=(k == 3),
                        )
                ob = ob_pool.tile([128, T], BF16, tag="ob")
                for n in range(4):
                    eng = nc.scalar if n < 2 else nc.vector
                    if n < 2:
                        nc.scalar.copy(ob[:, n * 512 : (n + 1) * 512], pn[n][:])
                    else:
                        nc.vector.tensor_copy(
                            ob[:, n * 512 : (n + 1) * 512], pn[n][:]
                        )
                nc.sync.dma_start(outT[m * 128 : (m + 1) * 128, :], ob[:])

    nc.compile()
    return nc


def _get_nc():
    if "nc" not in _CACHE:
        _CACHE["nc"] = _build()
    return _CACHE["nc"]


def _build_debug():
    return _build(debug=True)


def _prep_core_inputs(x, w_attn, b_attn, w_proj, b, g):
    cols = []
    for p in range(4):
        off = 512 * g + 128 * p
        cols += [
            w_attn[:, off : off + 128],
            w_attn[:, E + off : E + off + 128],
            w_attn[:, 2 * E + off : 2 * E + off + 128],
        ]
    wq = np.concatenate(cols, axis=1)  # [1024, 1536]
    wq = wq.reshape(8, 128, 12, 128).transpose(2, 1, 0, 3)  # [12, 128, 8, 128]
    wq = np.ascontiguousarray(wq, dtype=np.float32)
    bcols = []
    for p in range(4):
        off = 512 * g + 128 * p
        bcols += [
            b_attn[off : off + 128],
            b_attn[E + off : E + off + 128],
            b_attn[2 * E + off : 2 * E + off + 128],
        ]
    bq = np.stack(bcols, axis=1).astype(np.float32)  # [128, 12]
    wpr = np.concatenate(
        [w_proj[512 * g + 128 * p : 512 * g + 128 * p + 128, :] for p in range(4)],
        axis=0,
    )  # [512, 1024]
    wpr = np.ascontiguousarray(
        wpr.reshape(4, 128, 1024).transpose(1, 0, 2), dtype=np.float32
    )
    xT = np.ascontiguousarray(
        x[b].T.reshape(8, 128, T).transpose(1, 0, 2)
    )  # [128, 8, T]: [p, k, t] = x[b][t, 128k+p]
    return {
        "xin": xT.astype(ml_dtypes.bfloat16),
        "wqkv": wq.astype(ml_dtypes.bfloat16),
        "bqkv": np.ascontiguousarray(bq),
        "wp": wpr.astype(ml_dtypes.bfloat16),
    }


def kernel(x, w_attn, b_attn, w_proj, b_proj, _trace=False):
    from concourse.bass_utils import run_bass_kernel_spmd

    x = np.asarray(x, dtype=np.float32)
    w_attn = np.asarray(w_attn, dtype=np.float32)
    b_attn = np.asarray(b_attn, dtype=np.float32)
    w_proj = np.asarray(w_proj, dtype=np.float32)
    b_proj = np.asarray(b_proj, dtype=np.float32)

    nc = _get_nc()
    in_maps = [
        _prep_core_inputs(x, w_attn, b_attn, w_proj, core // 2, core % 2)
        for core in range(8)
    ]
    res = run_bass_kernel_spmd(nc, in_maps, core_ids=list(range(8)), trace=_trace)
    _CACHE["last_results"] = res
    out = np.empty((B, T, E), dtype=np.float32)
    for b in range(B):
        acc = res.results[2 * b]["outT"].astype(np.float32) + res.results[
            2 * b + 1
        ]["outT"].astype(np.float32)
        out[b] = acc.T + b_proj[None, :]
    return out


# revision 39
# speedup vs baseline: 1.2605x; 1.2605x over previous
"""Causal self-attention on 8 trn2 NeuronCores — v2.

Sharding: core = 2*b + g  (b in 0..3 batches, g in 0..1 head-groups of 8
heads). Host passes x^T per batch (so no on-chip transposes); per core:
  qkv^T = Wslice^T @ x^T   (feature-major), emitted interleaved with
  attention so the scalar engine's exp stream starts early.
  Attention in scores^T layout [k, q], both heads of a 128-partition
  group processed together (s=0 on PE rows 0:63, s=1 on rows 64:127).
  The AV lhsT is [V | ones] (128 cols), so the softmax denominator lands
  replicated on output partitions 64:128 — no cross-partition broadcast.
  Normalization: reciprocal_approx_fast + one multiply per (head, 1024 q).
  partial out^T = yT @ Wp_slice -> [1024, 2048], DMA'd from PSUM.
Host gathers: out[b] = (partial[2b] + partial[2b+1]).T + b_proj.
"""

import numpy as np
import ml_dtypes

B, T, E, H = 4, 2048, 1024, 16
HD = E // H  # 64

_CACHE = {}


def _build(debug=False):
    from contextlib import ExitStack

    import concourse.bass as bass
    import concourse.mybir as mybir
    import concourse.tile as tile
    from concourse import bacc

    F32 = mybir.dt.float32
    BF16 = mybir.dt.bfloat16
    AF = mybir.ActivationFunctionType
    ALU = mybir.AluOpType

    nc = bacc.Bacc("TRN2", target_bir_lowering=False)
    xin = nc.dram_tensor("xin", [128, 8, T], BF16, kind="ExternalInput")
    wqkv = nc.dram_tensor("wqkv", [12, 128, 8, 128], BF16, kind="ExternalInput")
    bqkv = nc.dram_tensor("bqkv", [128, 12], F32, kind="ExternalInput")
    wp = nc.dram_tensor("wp", [128, 4, 1024], BF16, kind="ExternalInput")
    outT = nc.dram_tensor("outT", [E, T], BF16, kind="ExternalOutput")
    if debug:
        qkvT_dbg = nc.dram_tensor(
            "qkvT_dbg", [128, 12, T], BF16, kind="ExternalOutput"
        )
        yT_dbg = nc.dram_tensor("yT_dbg", [128, 4, T], BF16, kind="ExternalOutput")

    with tile.TileContext(nc) as tc, ExitStack() as ctx:
        const = ctx.enter_context(tc.tile_pool(name="const", bufs=1))
        # stacked 64x64 identities at partitions 0 and 64 (v-transpose lhsT
        # sits at partition base 0 or 64)
        id2f = const.tile([128, 64], F32, tag="id2f")
        nc.gpsimd.memset(id2f[:], 0.0)
        for off in (0, 64):
            nc.gpsimd.affine_select(
                out=id2f[:],
                in_=id2f[:],
                compare_op=ALU.not_equal,
                fill=1.0,
                base=-off,
                pattern=[[-1, 64]],
                channel_multiplier=1,
            )
        id2 = const.tile([128, 64], BF16, tag="id2")
        nc.vector.tensor_copy(id2[:], id2f[:])
        from concourse.masks import make_identity

        identf = const.tile([128, 128], F32, tag="identf")
        make_identity(nc, identf[:])
        identr = const.tile([128, 128], BF16, tag="identr")
        nc.vector.tensor_copy(identr[:], identf[:])
        biasT = const.tile([128, 12], F32, tag="biasT")
        nc.sync.dma_start(biasT[:], bqkv[:])
        wps = const.tile([128, 4, 1024], BF16, tag="wps")
        ones64 = const.tile([128, 16, 64], BF16, tag="ones64")
        nc.gpsimd.memset(ones64[:], 1.0)
        # masks[j][p, col] = 1 iff col >= 128*j + p (causal keep in [k,q] layout)
        masks = []
        with tc.tile_pool(name="mtmp", bufs=2) as mtmp:
            for j in range(4):
                mjf = mtmp.tile([128, 512], F32, tag="maskf", name=f"maskf{j}")
                nc.gpsimd.memset(mjf[:], 1.0)
                nc.gpsimd.affine_select(
                    out=mjf[:],
                    in_=mjf[:],
                    compare_op=ALU.is_ge,
                    fill=0.0,
                    base=-128 * j,
                    pattern=[[1, 512]],
                    channel_multiplier=-1,
                )
                mj = const.tile([128, 512], BF16, tag=f"mask{j}", name=f"mask{j}")
                nc.vector.tensor_copy(mj[:], mjf[:])
                masks.append(mj)

        xT_pool = ctx.enter_context(tc.tile_pool(name="xT", bufs=1))
        xT = xT_pool.tile([128, 8, T], BF16, tag="xT")

        qkvT_pool = ctx.enter_context(tc.tile_pool(name="qkvT", bufs=1))
        qkvT = qkvT_pool.tile([128, 12, T], BF16, tag="qkvT")
        yT_pool = ctx.enter_context(tc.tile_pool(name="yT", bufs=1))
        yT = yT_pool.tile([128, 4, T], BF16, tag="yT")
        # vaug[:, s, kb, 0:64] = V block (k rows on partitions), cols 64:128
        # stay at the initial memset value 1.0 (denominator ones)


        with (
            tc.tile_pool(name="wq", bufs=4) as wq_pool,
            tc.tile_pool(name="vaug", bufs=2) as vaug_pool,
            tc.tile_pool(name="Pp", bufs=3) as P_pool,
            tc.tile_pool(name="rec", bufs=2) as rec_pool,
            tc.tile_pool(name="psBs", bufs=2, space="PSUM") as psBs,
            tc.tile_pool(name="psBy", bufs=2, space="PSUM") as psBy,
        ):
            # P buffers are partially overwritten each use; the mask multiply
            # reads the full 512 chunk, so zero them once up front.
            for s in range(2):
                for i in range(3):
                    Pt0 = P_pool.tile([128, 1024], BF16, tag=f"P{s}", name=f"Pt0_{s}_{i}")
                    nc.gpsimd.memset(Pt0[:], 0.0)

            # DMA order matters: the first QKV matmul needs wqm[0] and
            # xT[k=0] only — put them first so the PE starts ~2us in, and
            # defer wps (phase C) to the end of the input stream.
            wqms = [
                wq_pool.tile([128, 8, 128], BF16, tag="wqm", name=f"wqm{m}")
                for m in range(12)
            ]
            nc.sync.dma_start(wqms[0][:], wqkv[0])
            nc.sync.dma_start(wqms[1][:], wqkv[1])
            for k in range(8):
                nc.sync.dma_start(xT[:, k, :], xin[:, k, :])
            for m in range(2, 12):
                nc.sync.dma_start(wqms[m][:], wqkv[m])
            nc.sync.dma_start(wps[:], wp[:])

            def emit_a(m, jj):
                pq = psBs.tile([128, 1024], F32, tag="s", name=f"pq{m}_{jj}")
                for j2 in range(2):
                    for k in range(8):
                        nc.tensor.matmul(
                            pq[:, j2 * 512 : (j2 + 1) * 512],
                            wqms[m][:, k, :],
                            xT[:, k, (jj * 2 + j2) * 512 : (jj * 2 + j2 + 1) * 512],
                            start=(k == 0),
                            stop=(k == 7),
                        )
                nc.vector.tensor_scalar_add(
                    qkvT[:, m, jj * 1024 : (jj + 1) * 1024],
                    pq[:],
                    biasT[:, m : m + 1],
                )

            # A(0) up front; A(p+1) units are emitted at the two qc
            # boundaries inside p's attention — full-array QKV bursts there
            # re-warm the PE's activity monitor and fill the ymm-recycle wait.
            for m in range(3):
                for jj in range(2):
                    emit_a(m, jj)

            for p in range(4):
                a_units = (
                    [(m, jj) for m in range(3 * p + 3, 3 * p + 6) for jj in range(2)]
                    if p < 3
                    else []
                )

                # ---- V transposes into vaug [k, V(64)|ones(64)] per kb.
                # One matmul per kb transposes BOTH heads (contraction over
                # all 128 dim-partitions, identity rhs), keeping N=128 and
                # the PE duty cycle high enough not to trip the HAM throttle.
                vaug = vaug_pool.tile([128, 2, 2048], BF16, tag="vaug")
                vaug_k = vaug[:, :, :].rearrange("p s (kb c) -> p s kb c", kb=16, c=128)
                for s in range(2):
                    nc.vector.tensor_copy(vaug_k[:, s, :, 64:128], ones64[:])
                for h in range(2):
                    pvt = psBs.tile([128, 1024], F32, tag="s")
                    for i in range(8):
                        kb = 8 * h + i
                        nc.tensor.matmul(
                            pvt[:, i * 128 : (i + 1) * 128],
                            qkvT[:, 3 * p + 2, kb * 128 : (kb + 1) * 128],
                            identr[:],
                            start=True,
                            stop=True,
                        )
                    pvt_k = pvt[:, :].rearrange("p (a c) -> p a c", a=8, c=128)
                    for s in range(2):
                        nc.vector.tensor_copy(
                            vaug_k[:, s, 8 * h : 8 * h + 8, 0:64],
                            pvt_k[:, :, 64 * s : 64 * s + 64],
                        )

                # ---- attention for heads (p, s=0), (p, s=1), s-fused ----
                # scores for the two heads are row-packed: s=0 weights occupy
                # PE rows 0:63, s=1 rows 64:127 (disjoint row groups -> the
                # two matmuls run concurrently and keep the full array busy)
                qT = [qkvT[64 * s : 64 * s + 64, 3 * p, :] for s in range(2)]
                kT = [qkvT[64 * s : 64 * s + 64, 3 * p + 1, :] for s in range(2)]
                for qc in range(2):
                    q0 = qc * 1024
                    kmax = (qc + 1) * 8
                    klast = [
                        min(kmax, qc * 8 + (ci + 1) * 4) - 1 for ci in range(2)
                    ]
                    ymm = [
                        psBy.tile([128, 1024], F32, tag="ymm", name=f"ymm{p}_{qc}_{s}")
                        for s in range(2)
                    ]
                    for kb in range(kmax):
                        w0 = max(0, kb * 128 - q0)  # exact causal offset
                        c0 = (w0 // 512) * 512  # 512-aligned chunk start
                        sp = [
                            psBs.tile([128, 1024], F32, tag="s", name=f"sp{p}_{qc}_{kb}_{s}")
                            for s in range(2)
                        ]
                        for j in range(c0, 1024, 512):
                            for s in range(2):
                                nc.tensor.matmul(
                                    sp[s][:, j : j + 512],
                                    kT[s][:, kb * 128 : (kb + 1) * 128],
                                    qT[s][:, q0 + j : q0 + j + 512],
                                    start=True,
                                    stop=True,
                                    tile_position=(64 * s, 0),
                                )
                        pts = []
                        for s in range(2):
                            Pt = P_pool.tile(
                                [128, 1024], BF16, tag=f"P{s}", name=f"Pt{p}_{qc}_{kb}_{s}"
                            )
                            nc.scalar.activation(
                                Pt[:, w0:1024], sp[s][:, w0:1024], AF.Exp, scale=0.125
                            )
                            if kb >= qc * 8:  # diagonal block: causal mask
                                nc.vector.tensor_tensor(
                                    out=Pt[:, c0 : c0 + 512],
                                    in0=Pt[:, c0 : c0 + 512],
                                    in1=masks[(w0 - c0) // 128][:],
                                    op=ALU.mult,
                                )
                            pts.append(Pt)
                        for j in range(c0, 1024, 512):
                            for s in range(2):
                                nc.tensor.matmul(
                                    ymm[s][:, j : j + 512],
                                    vaug[:, s, kb * 128 : (kb + 1) * 128],
                                    pts[s][:, j : j + 512],
                                    start=(kb == 0),
                                    stop=(kb == klast[j // 512]),
                                )
                    for s in range(2):
                        den = rec_pool.tile([64, 1024], F32, tag="den")
                        nc.vector.tensor_copy(den[:], ymm[s][64:128, :])
                        rec = rec_pool.tile([64, 1024], F32, tag="rec")
                        nc.vector.reciprocal_approx_fast(rec[:], den[:])
                        nc.vector.tensor_tensor(
                            out=yT[64 * s : 64 * s + 64, p, q0 : q0 + 1024],
                            in0=ymm[s][0:64, :],
                            in1=rec[:],
                            op=ALU.mult,
                        )
                    for u in a_units[qc * 3 : qc * 3 + 3]:
                        emit_a(*u)

        if debug:
            nc.sync.dma_start(qkvT_dbg[:], qkvT[:])
            nc.sync.dma_start(yT_dbg[:], yT[:])

        with (
            tc.tile_pool(name="ob", bufs=1) as ob_pool,
            tc.tile_pool(name="psC", bufs=8, space="PSUM") as psC,
        ):
            for m in range(8):
                pn = [
                    psC.tile([128, 512], F32, tag="pc", name=f"pc{m}_{n}")
                    for n in range(4)
                ]
                for k in range(4):
                    for n in range(4):
                        nc.tensor.matmul(
                            pn[n][:],
                            wps[:, k, m * 128 : (m + 1) * 128],
                            yT[:, k, n * 512 : (n + 1) * 512],
                            start=(k == 0),
                            stop=(k == 3),
                        )
                ob = ob_pool.tile([128, T], BF16, tag="ob")
                for n in range(4):
                    eng = nc.scalar if n < 2 else nc.vector
                    if n < 2:
                        nc.scalar.copy(ob[:, n * 512 : (n + 1) * 512], pn[n][:])
                    else:
                        nc.vector.tensor_copy(
                            ob[:, n * 512 : (n + 1) * 512], pn[n][:]
                        )
                nc.sync.dma_start(outT[m * 128 : (m + 1) * 128, :], ob[:])

    nc.compile()
    return nc


def _get_nc():
    if "nc" not in _CACHE:
        _CACHE["nc"] = _build()
    return _CACHE["nc"]


def _build_debug():
    return _build(debug=True)


def _prep_core_inputs(x, w_attn, b_attn, w_proj, b, g):
    cols = []
    for p in range(4):
        off = 512 * g + 128 * p
        cols += [
            w_attn[:, off : off + 128],
            w_attn[:, E + off : E + off + 128],
            w_attn[:, 2 * E + off : 2 * E + off + 128],
        ]
    wq = np.concatenate(cols, axis=1)  # [1024, 1536]
    wq = wq.reshape(8, 128, 12, 128).transpose(2, 1, 0, 3)  # [12, 128, 8, 128]
    wq = np.ascontiguousarray(wq, dtype=np.float32)
    bcols = []
    for p in range(4):
        off = 512 * g + 128 * p
        bcols += [
            b_attn[off : off + 128],
            b_attn[E + off : E + off + 128],
            b_attn[2 * E + off : 2 * E + off + 128],
        ]
    bq = np.stack(bcols, axis=1).astype(np.float32)  # [128, 12]
    wpr = np.concatenate(
        [w_proj[512 * g + 128 * p : 512 * g + 128 * p + 128, :] for p in range(4)],
        axis=0,
    )  # [512, 1024]
    wpr = np.ascontiguousarray(
        wpr.reshape(4, 128, 1024).transpose(1, 0, 2), dtype=np.float32
    )
    xT = np.ascontiguousarray(
        x[b].T.reshape(8, 128, T).transpose(1, 0, 2)
    )  # [128, 8, T]: [p, k, t] = x[b][t, 128k+p]
    return {
        "xin": xT.astype(ml_dtypes.bfloat16),
        "wqkv": wq.astype(ml_dtypes.bfloat16),
        "bqkv": np.ascontiguousarray(bq),
        "wp": wpr.astype(ml_dtypes.bfloat16),
    }


def kernel(x, w_attn, b_attn, w_proj, b_proj, _trace=False):
    from concourse.bass_utils import run_bass_kernel_spmd

    x = np.asarray(x, dtype=np.float32)
    w_attn = np.asarray(w_attn, dtype=np.float32)
    b_attn = np.asarray(b_attn, dtype=np.float32)
    w_proj = np.asarray(w_proj, dtype=np.float32)
    b_proj = np.asarray(b_proj, dtype=np.float32)

    nc = _get_nc()
    in_maps = [
        _prep_core_inputs(x, w_attn, b_attn, w_proj, core // 2, core % 2)
        for core in range(8)
    ]
    res = run_bass_kernel_spmd(nc, in_maps, core_ids=list(range(8)), trace=_trace)
    _CACHE["last_results"] = res
    out = np.empty((B, T, E), dtype=np.float32)
    for b in range(B):
        acc = res.results[2 * b]["outT"].astype(np.float32) + res.results[
            2 * b + 1
        ]["outT"].astype(np.float32)
        out[b] = acc.T + b_proj[None, :]
    return out


# revision 43
# speedup vs baseline: 1.4290x; 1.1336x over previous
"""Causal self-attention on 8 trn2 NeuronCores — v2.

Sharding: core = 2*b + g  (b in 0..3 batches, g in 0..1 head-groups of 8
heads). Host passes x^T per batch (so no on-chip transposes); per core:
  qkv^T = Wslice^T @ x^T   (feature-major), emitted interleaved with
  attention so the scalar engine's exp stream starts early.
  Attention in scores^T layout [k, q], both heads of a 128-partition
  group processed together (s=0 on PE rows 0:63, s=1 on rows 64:127).
  The AV lhsT is [V | ones] (128 cols), so the softmax denominator lands
  replicated on output partitions 64:128 — no cross-partition broadcast.
  Normalization: reciprocal_approx_fast + one multiply per (head, 1024 q).
  partial out^T = yT @ Wp_slice -> [1024, 2048], DMA'd from PSUM.
Host gathers: out[b] = (partial[2b] + partial[2b+1]).T + b_proj.
"""

import numpy as np
import ml_dtypes

B, T, E, H = 4, 2048, 1024, 16
HD = E // H  # 64

_CACHE = {}


def _build(debug=False):
    from contextlib import ExitStack

    import concourse.bass as bass
    import concourse.mybir as mybir
    import concourse.tile as tile
    from concourse import bacc

    F32 = mybir.dt.float32
    BF16 = mybir.dt.bfloat16
    AF = mybir.ActivationFunctionType
    ALU = mybir.AluOpType

    nc = bacc.Bacc("TRN2", target_bir_lowering=False)
    xin = nc.dram_tensor("xin", [128, 8, T], BF16, kind="ExternalInput")
    wqkv = nc.dram_tensor("wqkv", [12, 128, 8, 128], BF16, kind="ExternalInput")
    bqkv = nc.dram_tensor("bqkv", [128, 12], F32, kind="ExternalInput")
    wp = nc.dram_tensor("wp", [128, 4, 1024], BF16, kind="ExternalInput")
    outT = nc.dram_tensor("outT", [E, T], BF16, kind="ExternalOutput")
    if debug:
        qkvT_dbg = nc.dram_tensor(
            "qkvT_dbg", [128, 12, T], BF16, kind="ExternalOutput"
        )
        yT_dbg = nc.dram_tensor("yT_dbg", [128, 4, T], BF16, kind="ExternalOutput")

    with tile.TileContext(nc) as tc, ExitStack() as ctx:
        const = ctx.enter_context(tc.tile_pool(name="const", bufs=1))
        # stacked 64x64 identities at partitions 0 and 64 (v-transpose lhsT
        # sits at partition base 0 or 64)
        id2f = const.tile([128, 64], F32, tag="id2f")
        nc.gpsimd.memset(id2f[:], 0.0)
        for off in (0, 64):
            nc.gpsimd.affine_select(
                out=id2f[:],
                in_=id2f[:],
                compare_op=ALU.not_equal,
                fill=1.0,
                base=-off,
                pattern=[[-1, 64]],
                channel_multiplier=1,
            )
        id2 = const.tile([128, 64], BF16, tag="id2")
        nc.vector.tensor_copy(id2[:], id2f[:])
        from concourse.masks import make_identity

        identf = const.tile([128, 128], F32, tag="identf")
        make_identity(nc, identf[:])
        identr = const.tile([128, 128], BF16, tag="identr")
        nc.vector.tensor_copy(identr[:], identf[:])
        biasT = const.tile([128, 12], F32, tag="biasT")
        nc.sync.dma_start(biasT[:], bqkv[:])
        wps = const.tile([128, 4, 1024], BF16, tag="wps")
        ones64 = const.tile([128, 16, 64], BF16, tag="ones64")
        nc.gpsimd.memset(ones64[:], 1.0)
        # masks[j][p, col] = 1 iff col >= 128*j + p (causal keep in [k,q] layout)
        masks = []
        with tc.tile_pool(name="mtmp", bufs=2) as mtmp:
            for j in range(4):
                mjf = mtmp.tile([128, 512], F32, tag="maskf", name=f"maskf{j}")
                nc.gpsimd.memset(mjf[:], 1.0)
                nc.gpsimd.affine_select(
                    out=mjf[:],
                    in_=mjf[:],
                    compare_op=ALU.is_ge,
                    fill=0.0,
                    base=-128 * j,
                    pattern=[[1, 512]],
                    channel_multiplier=-1,
                )
                mj = const.tile([128, 512], BF16, tag=f"mask{j}", name=f"mask{j}")
                nc.vector.tensor_copy(mj[:], mjf[:])
                masks.append(mj)

        xT_pool = ctx.enter_context(tc.tile_pool(name="xT", bufs=1))
        xT = xT_pool.tile([128, 8, T], BF16, tag="xT")

        qkvT_pool = ctx.enter_context(tc.tile_pool(name="qkvT", bufs=1))
        qkvT = qkvT_pool.tile([128, 12, T], BF16, tag="qkvT")
        yT_pool = ctx.enter_context(tc.tile_pool(name="yT", bufs=1))
        yT = yT_pool.tile([128, 4, T], BF16, tag="yT")
        # zero-padded per-head K: scores then run with full 128-row
        # contraction (the zero rows null the other head's q), which keeps
        # the PE activity monitor warm through the attention phase.
        kpad_pool = ctx.enter_context(tc.tile_pool(name="kpad", bufs=1))
        kpad = kpad_pool.tile([128, 2, T], BF16, tag="kpad")
        nc.gpsimd.memset(kpad[:], 0.0)
        # vaug[:, s, kb, 0:64] = V block (k rows on partitions), cols 64:128
        # stay at the initial memset value 1.0 (denominator ones)


        with (
            tc.tile_pool(name="wq", bufs=4) as wq_pool,
            tc.tile_pool(name="vaug", bufs=2) as vaug_pool,
            tc.tile_pool(name="Pp", bufs=3) as P_pool,
            tc.tile_pool(name="rec", bufs=2) as rec_pool,
            tc.tile_pool(name="psBs", bufs=2, space="PSUM") as psBs,
            tc.tile_pool(name="psBy", bufs=2, space="PSUM") as psBy,
        ):
            # P buffers are partially overwritten each use; the mask multiply
            # reads the full 512 chunk, so zero them once up front.
            for s in range(2):
                for i in range(3):
                    Pt0 = P_pool.tile([128, 1024], BF16, tag=f"P{s}", name=f"Pt0_{s}_{i}")
                    nc.gpsimd.memset(Pt0[:], 0.0)

            # DMA order matters: the first QKV matmul needs wqm[0] and
            # xT[k=0] only — put them first so the PE starts ~2us in, and
            # defer wps (phase C) to the end of the input stream.
            wqms = [
                wq_pool.tile([128, 8, 128], BF16, tag="wqm", name=f"wqm{m}")
                for m in range(12)
            ]
            nc.sync.dma_start(wqms[0][:], wqkv[0])
            nc.sync.dma_start(wqms[1][:], wqkv[1])
            for k in range(8):
                nc.sync.dma_start(xT[:, k, :], xin[:, k, :])
            for m in range(2, 12):
                nc.sync.dma_start(wqms[m][:], wqkv[m])
            nc.sync.dma_start(wps[:], wp[:])

            for p in range(4):
                # ---- phase A chunk: qkv^T columns m = 3p..3p+2 ----
                for m in range(3 * p, 3 * p + 3):
                    for jj in range(2):
                        pq = psBs.tile([128, 1024], F32, tag="s", name=f"pq{m}_{jj}")
                        for j2 in range(2):
                            for k in range(8):
                                nc.tensor.matmul(
                                    pq[:, j2 * 512 : (j2 + 1) * 512],
                                    wqms[m][:, k, :],
                                    xT[
                                        :,
                                        k,
                                        (jj * 2 + j2) * 512 : (jj * 2 + j2 + 1) * 512,
                                    ],
                                    start=(k == 0),
                                    stop=(k == 7),
                                )
                        nc.vector.tensor_scalar_add(
                            qkvT[:, m, jj * 1024 : (jj + 1) * 1024],
                            pq[:],
                            biasT[:, m : m + 1],
                        )

                # ---- V transposes into vaug [k, V(64)|ones(64)] per kb.
                # One matmul per kb transposes BOTH heads (contraction over
                # all 128 dim-partitions, identity rhs), keeping N=128 and
                # the PE duty cycle high enough not to trip the HAM throttle.
                vaug = vaug_pool.tile([128, 2, 2048], BF16, tag="vaug")
                vaug_k = vaug[:, :, :].rearrange("p s (kb c) -> p s kb c", kb=16, c=128)
                for s in range(2):
                    nc.vector.tensor_copy(vaug_k[:, s, :, 64:128], ones64[:])
                for h in range(2):
                    pvt = psBs.tile([128, 1024], F32, tag="s")
                    for i in range(8):
                        kb = 8 * h + i
                        nc.tensor.matmul(
                            pvt[:, i * 128 : (i + 1) * 128],
                            qkvT[:, 3 * p + 2, kb * 128 : (kb + 1) * 128],
                            identr[:],
                            start=True,
                            stop=True,
                        )
                    pvt_k = pvt[:, :].rearrange("p (a c) -> p a c", a=8, c=128)
                    for s in range(2):
                        nc.vector.tensor_copy(
                            vaug_k[:, s, 8 * h : 8 * h + 8, 0:64],
                            pvt_k[:, :, 64 * s : 64 * s + 64],
                        )

                # ---- attention for heads (p, s=0), (p, s=1), s-fused ----
                for s in range(2):
                    nc.vector.tensor_copy(
                        kpad[64 * s : 64 * s + 64, s, :],
                        qkvT[64 * s : 64 * s + 64, 3 * p + 1, :],
                    )
                qTf = qkvT[:, 3 * p, :]
                for qc in range(2):
                    q0 = qc * 1024
                    kmax = (qc + 1) * 8
                    klast = [
                        min(kmax, qc * 8 + (ci + 1) * 4) - 1 for ci in range(2)
                    ]
                    ymm = [
                        psBy.tile([128, 1024], F32, tag="ymm", name=f"ymm{p}_{qc}_{s}")
                        for s in range(2)
                    ]
                    for kb in range(kmax):
                        w0 = max(0, kb * 128 - q0)  # exact causal offset
                        c0 = (w0 // 512) * 512  # 512-aligned chunk start
                        sp = [
                            psBs.tile([128, 1024], F32, tag="s", name=f"sp{p}_{qc}_{kb}_{s}")
                            for s in range(2)
                        ]
                        for j in range(c0, 1024, 512):
                            for s in range(2):
                                nc.tensor.matmul(
                                    sp[s][:, j : j + 512],
                                    kpad[:, s, kb * 128 : (kb + 1) * 128],
                                    qTf[:, q0 + j : q0 + j + 512],
                                    start=True,
                                    stop=True,
                                )
                        pts = []
                        for s in range(2):
                            Pt = P_pool.tile(
                                [128, 1024], BF16, tag=f"P{s}", name=f"Pt{p}_{qc}_{kb}_{s}"
                            )
                            nc.scalar.activation(
                                Pt[:, w0:1024], sp[s][:, w0:1024], AF.Exp, scale=0.125
                            )
                            if kb >= qc * 8:  # diagonal block: causal mask
                                nc.vector.tensor_tensor(
                                    out=Pt[:, c0 : c0 + 512],
                                    in0=Pt[:, c0 : c0 + 512],
                                    in1=masks[(w0 - c0) // 128][:],
                                    op=ALU.mult,
                                )
                            pts.append(Pt)
                        for j in range(c0, 1024, 512):
                            for s in range(2):
                                nc.tensor.matmul(
                                    ymm[s][:, j : j + 512],
                                    vaug[:, s, kb * 128 : (kb + 1) * 128],
                                    pts[s][:, j : j + 512],
                                    start=(kb == 0),
                                    stop=(kb == klast[j // 512]),
                                )
                    for s in range(2):
                        den = rec_pool.tile([64, 1024], F32, tag="den")
                        nc.vector.tensor_copy(den[:], ymm[s][64:128, :])
                        rec = rec_pool.tile([64, 1024], F32, tag="rec")
                        nc.vector.reciprocal_approx_fast(rec[:], den[:])
                        nc.vector.tensor_tensor(
                            out=yT[64 * s : 64 * s + 64, p, q0 : q0 + 1024],
                            in0=ymm[s][0:64, :],
                            in1=rec[:],
                            op=ALU.mult,
                        )

        if debug:
            nc.sync.dma_start(qkvT_dbg[:], qkvT[:])
            nc.sync.dma_start(yT_dbg[:], yT[:])

        with (
            tc.tile_pool(name="ob", bufs=1) as ob_pool,
            tc.tile_pool(name="psC", bufs=8, space="PSUM") as psC,
        ):
            for m in range(8):
                pn = [
                    psC.tile([128, 512], F32, tag="pc", name=f"pc{m}_{n}")
                    for n in range(4)
                ]
                for k in range(4):
                    for n in range(4):
                        nc.tensor.matmul(
                            pn[n][:],
                            wps[:, k, m * 128 : (m + 1) * 128],
                            yT[:, k, n * 512 : (n + 1) * 512],
                            start=(k == 0),
                            stop=(k == 3),
                        )
                ob = ob_pool.tile([128, T], BF16, tag="ob")
                for n in range(4):
                    eng = nc.scalar if n < 2 else nc.vector
                    if n < 2:
                        nc.scalar.copy(ob[:, n * 512 : (n + 1) * 512], pn[n][:])
                    else:
                        nc.vector.tensor_copy(
                            ob[:, n * 512 : (n + 1) * 512], pn[n][:]
                        )
                nc.sync.dma_start(outT[m * 128 : (m + 1) * 128, :], ob[:])

    nc.compile()
    return nc


def _get_nc():
    if "nc" not in _CACHE:
        _CACHE["nc"] = _build()
    return _CACHE["nc"]


def _build_debug():
    return _build(debug=True)


def _prep_core_inputs(x, w_attn, b_attn, w_proj, b, g):
    cols = []
    for p in range(4):
        off = 512 * g + 128 * p
        cols += [
            w_attn[:, off : off + 128],
            w_attn[:, E + off : E + off + 128],
            w_attn[:, 2 * E + off : 2 * E + off + 128],
        ]
    wq = np.concatenate(cols, axis=1)  # [1024, 1536]
    wq = wq.reshape(8, 128, 12, 128).transpose(2, 1, 0, 3)  # [12, 128, 8, 128]
    wq = np.ascontiguousarray(wq, dtype=np.float32)
    bcols = []
    for p in range(4):
        off = 512 * g + 128 * p
        bcols += [
            b_attn[off : off + 128],
            b_attn[E + off : E + off + 128],
            b_attn[2 * E + off : 2 * E + off + 128],
        ]
    bq = np.stack(bcols, axis=1).astype(np.float32)  # [128, 12]
    wpr = np.concatenate(
        [w_proj[512 * g + 128 * p : 512 * g + 128 * p + 128, :] for p in range(4)],
        axis=0,
    )  # [512, 1024]
    wpr = np.ascontiguousarray(
        wpr.reshape(4, 128, 1024).transpose(1, 0, 2), dtype=np.float32
    )
    xT = np.ascontiguousarray(
        x[b].T.reshape(8, 128, T).transpose(1, 0, 2)
    )  # [128, 8, T]: [p, k, t] = x[b][t, 128k+p]
    return {
        "xin": xT.astype(ml_dtypes.bfloat16),
        "wqkv": wq.astype(ml_dtypes.bfloat16),
        "bqkv": np.ascontiguousarray(bq),
        "wp": wpr.astype(ml_dtypes.bfloat16),
    }


def kernel(x, w_attn, b_attn, w_proj, b_proj, _trace=False):
    from concourse.bass_utils import run_bass_kernel_spmd

    x = np.asarray(x, dtype=np.float32)
    w_attn = np.asarray(w_attn, dtype=np.float32)
    b_attn = np.asarray(b_attn, dtype=np.float32)
    w_proj = np.asarray(w_proj, dtype=np.float32)
    b_proj = np.asarray(b_proj, dtype=np.float32)

    nc = _get_nc()
    in_maps = [
        _prep_core_inputs(x, w_attn, b_attn, w_proj, core // 2, core % 2)
        for core in range(8)
    ]
    res = run_bass_kernel_spmd(nc, in_maps, core_ids=list(range(8)), trace=_trace)
    _CACHE["last_results"] = res
    out = np.empty((B, T, E), dtype=np.float32)
    for b in range(B):
        acc = res.results[2 * b]["outT"].astype(np.float32) + res.results[
            2 * b + 1
        ]["outT"].astype(np.float32)
        out[b] = acc.T + b_proj[None, :]
    return out


# revision 44
# speedup vs baseline: 1.4361x; 1.0050x over previous
"""Causal self-attention on 8 trn2 NeuronCores — v2.

Sharding: core = 2*b + g  (b in 0..3 batches, g in 0..1 head-groups of 8
heads). Host passes x^T per batch (so no on-chip transposes); per core:
  qkv^T = Wslice^T @ x^T   (feature-major), emitted interleaved with
  attention so the scalar engine's exp stream starts early.
  Attention in scores^T layout [k, q], both heads of a 128-partition
  group processed together (s=0 on PE rows 0:63, s=1 on rows 64:127).
  The AV lhsT is [V | ones] (128 cols), so the softmax denominator lands
  replicated on output partitions 64:128 — no cross-partition broadcast.
  Normalization: reciprocal_approx_fast + one multiply per (head, 1024 q).
  partial out^T = yT @ Wp_slice -> [1024, 2048], DMA'd from PSUM.
Host gathers: out[b] = (partial[2b] + partial[2b+1]).T + b_proj.
"""

import numpy as np
import ml_dtypes

B, T, E, H = 4, 2048, 1024, 16
HD = E // H  # 64

_CACHE = {}


def _build(debug=False):
    from contextlib import ExitStack

    import concourse.bass as bass
    import concourse.mybir as mybir
    import concourse.tile as tile
    from concourse import bacc

    F32 = mybir.dt.float32
    BF16 = mybir.dt.bfloat16
    AF = mybir.ActivationFunctionType
    ALU = mybir.AluOpType

    nc = bacc.Bacc("TRN2", target_bir_lowering=False)
    xin = nc.dram_tensor("xin", [128, 8, T], BF16, kind="ExternalInput")
    wqkv = nc.dram_tensor("wqkv", [12, 128, 8, 128], BF16, kind="ExternalInput")
    bqkv = nc.dram_tensor("bqkv", [128, 12], F32, kind="ExternalInput")
    wp = nc.dram_tensor("wp", [128, 4, 1024], BF16, kind="ExternalInput")
    outT = nc.dram_tensor("outT", [E, T], BF16, kind="ExternalOutput")
    if debug:
        qkvT_dbg = nc.dram_tensor(
            "qkvT_dbg", [128, 12, T], BF16, kind="ExternalOutput"
        )
        yT_dbg = nc.dram_tensor("yT_dbg", [128, 4, T], BF16, kind="ExternalOutput")

    with tile.TileContext(nc) as tc, ExitStack() as ctx:
        const = ctx.enter_context(tc.tile_pool(name="const", bufs=1))
        # stacked 64x64 identities at partitions 0 and 64 (v-transpose lhsT
        # sits at partition base 0 or 64)
        id2f = const.tile([128, 64], F32, tag="id2f")
        nc.gpsimd.memset(id2f[:], 0.0)
        for off in (0, 64):
            nc.gpsimd.affine_select(
                out=id2f[:],
                in_=id2f[:],
                compare_op=ALU.not_equal,
                fill=1.0,
                base=-off,
                pattern=[[-1, 64]],
                channel_multiplier=1,
            )
        id2 = const.tile([128, 64], BF16, tag="id2")
        nc.vector.tensor_copy(id2[:], id2f[:])
        from concourse.masks import make_identity

        identf = const.tile([128, 128], F32, tag="identf")
        make_identity(nc, identf[:])
        identr = const.tile([128, 128], BF16, tag="identr")
        nc.vector.tensor_copy(identr[:], identf[:])
        biasT = const.tile([128, 12], F32, tag="biasT")
        nc.sync.dma_start(biasT[:], bqkv[:])
        wps = const.tile([128, 4, 1024], BF16, tag="wps")
        ones64 = const.tile([128, 16, 64], BF16, tag="ones64")
        nc.gpsimd.memset(ones64[:], 1.0)
        # masks[j][p, col] = 1 iff col >= 128*j + p (causal keep in [k,q] layout)
        masks = []
        with tc.tile_pool(name="mtmp", bufs=2) as mtmp:
            for j in range(4):
                mjf = mtmp.tile([128, 512], F32, tag="maskf", name=f"maskf{j}")
                nc.gpsimd.memset(mjf[:], 1.0)
                nc.gpsimd.affine_select(
                    out=mjf[:],
                    in_=mjf[:],
                    compare_op=ALU.is_ge,
                    fill=0.0,
                    base=-128 * j,
                    pattern=[[1, 512]],
                    channel_multiplier=-1,
                )
                mj = const.tile([128, 512], BF16, tag=f"mask{j}", name=f"mask{j}")
                nc.vector.tensor_copy(mj[:], mjf[:])
                masks.append(mj)

        xT_pool = ctx.enter_context(tc.tile_pool(name="xT", bufs=1))
        xT = xT_pool.tile([128, 8, T], BF16, tag="xT")

        qkvT_pool = ctx.enter_context(tc.tile_pool(name="qkvT", bufs=1))
        qkvT = qkvT_pool.tile([128, 12, T], BF16, tag="qkvT")
        yT_pool = ctx.enter_context(tc.tile_pool(name="yT", bufs=1))
        yT = yT_pool.tile([128, 4, T], BF16, tag="yT")
        # zero-padded per-head K: scores then run with full 128-row
        # contraction (the zero rows null the other head's q), which keeps
        # the PE activity monitor warm through the attention phase.
        kpad_pool = ctx.enter_context(tc.tile_pool(name="kpad", bufs=1))
        kpads = []
        for i in range(2):
            kp = kpad_pool.tile([128, 2, T], BF16, tag=f"kpad{i}", name=f"kpad{i}")
            nc.gpsimd.memset(kp[:], 0.0)
            kpads.append(kp)
        # vaug[:, s, kb, 0:64] = V block (k rows on partitions), cols 64:128
        # stay at the initial memset value 1.0 (denominator ones)


        with (
            tc.tile_pool(name="wq", bufs=4) as wq_pool,
            tc.tile_pool(name="vaug", bufs=2) as vaug_pool,
            tc.tile_pool(name="Pp", bufs=3) as P_pool,
            tc.tile_pool(name="rec", bufs=2) as rec_pool,
            tc.tile_pool(name="psBs", bufs=2, space="PSUM") as psBs,
            tc.tile_pool(name="psBy", bufs=2, space="PSUM") as psBy,
        ):
            # P buffers are partially overwritten each use; the mask multiply
            # reads the full 512 chunk, so zero them once up front.
            for s in range(2):
                for i in range(3):
                    Pt0 = P_pool.tile([128, 1024], BF16, tag=f"P{s}", name=f"Pt0_{s}_{i}")
                    nc.gpsimd.memset(Pt0[:], 0.0)

            # DMA order matters: the first QKV matmul needs wqm[0] and
            # xT[k=0] only — put them first so the PE starts ~2us in, and
            # defer wps (phase C) to the end of the input stream.
            wqms = [
                wq_pool.tile([128, 8, 128], BF16, tag="wqm", name=f"wqm{m}")
                for m in range(12)
            ]
            nc.sync.dma_start(wqms[0][:], wqkv[0])
            nc.sync.dma_start(wqms[1][:], wqkv[1])
            for k in range(8):
                nc.sync.dma_start(xT[:, k, :], xin[:, k, :])
            for m in range(2, 12):
                nc.sync.dma_start(wqms[m][:], wqkv[m])
            nc.sync.dma_start(wps[:], wp[:])

            for p in range(4):
                # ---- phase A chunk: qkv^T columns m = 3p..3p+2 ----
                for m in range(3 * p, 3 * p + 3):
                    for jj in range(2):
                        pq = psBs.tile([128, 1024], F32, tag="s", name=f"pq{m}_{jj}")
                        for j2 in range(2):
                            for k in range(8):
                                nc.tensor.matmul(
                                    pq[:, j2 * 512 : (j2 + 1) * 512],
                                    wqms[m][:, k, :],
                                    xT[
                                        :,
                                        k,
                                        (jj * 2 + j2) * 512 : (jj * 2 + j2 + 1) * 512,
                                    ],
                                    start=(k == 0),
                                    stop=(k == 7),
                                )
                        nc.vector.tensor_scalar_add(
                            qkvT[:, m, jj * 1024 : (jj + 1) * 1024],
                            pq[:],
                            biasT[:, m : m + 1],
                        )

                # ---- V transposes into vaug [k, V(64)|ones(64)] per kb.
                # One matmul per kb transposes BOTH heads (contraction over
                # all 128 dim-partitions, identity rhs), keeping N=128 and
                # the PE duty cycle high enough not to trip the HAM throttle.
                vaug = vaug_pool.tile([128, 2, 2048], BF16, tag="vaug")
                vaug_k = vaug[:, :, :].rearrange("p s (kb c) -> p s kb c", kb=16, c=128)
                for s in range(2):
                    nc.vector.tensor_copy(vaug_k[:, s, :, 64:128], ones64[:])
                for h in range(2):
                    pvt = psBs.tile([128, 1024], F32, tag="s")
                    for i in range(8):
                        kb = 8 * h + i
                        nc.tensor.matmul(
                            pvt[:, i * 128 : (i + 1) * 128],
                            qkvT[:, 3 * p + 2, kb * 128 : (kb + 1) * 128],
                            identr[:],
                            start=True,
                            stop=True,
                        )
                    pvt_k = pvt[:, :].rearrange("p (a c) -> p a c", a=8, c=128)
                    for s in range(2):
                        nc.vector.tensor_copy(
                            vaug_k[:, s, 8 * h : 8 * h + 8, 0:64],
                            pvt_k[:, :, 64 * s : 64 * s + 64],
                        )

                # ---- attention for heads (p, s=0), (p, s=1), s-fused ----
                kpad = kpads[p % 2]
                for s in range(2):
                    nc.vector.tensor_copy(
                        kpad[64 * s : 64 * s + 64, s, :],
                        qkvT[64 * s : 64 * s + 64, 3 * p + 1, :],
                    )
                qTf = qkvT[:, 3 * p, :]
                for qc in range(2):
                    q0 = qc * 1024
                    kmax = (qc + 1) * 8
                    klast = [
                        min(kmax, qc * 8 + (ci + 1) * 4) - 1 for ci in range(2)
                    ]
                    ymm = [
                        psBy.tile([128, 1024], F32, tag="ymm", name=f"ymm{p}_{qc}_{s}")
                        for s in range(2)
                    ]
                    for kb in range(kmax):
                        w0 = max(0, kb * 128 - q0)  # exact causal offset
                        c0 = (w0 // 512) * 512  # 512-aligned chunk start
                        sp = [
                            psBs.tile([128, 1024], F32, tag="s", name=f"sp{p}_{qc}_{kb}_{s}")
                            for s in range(2)
                        ]
                        for j in range(c0, 1024, 512):
                            for s in range(2):
                                nc.tensor.matmul(
                                    sp[s][:, j : j + 512],
                                    kpad[:, s, kb * 128 : (kb + 1) * 128],
                                    qTf[:, q0 + j : q0 + j + 512],
                                    start=True,
                                    stop=True,
                                )
                        pts = []
                        for s in range(2):
                            Pt = P_pool.tile(
                                [128, 1024], BF16, tag=f"P{s}", name=f"Pt{p}_{qc}_{kb}_{s}"
                            )
                            nc.scalar.activation(
                                Pt[:, w0:1024], sp[s][:, w0:1024], AF.Exp, scale=0.125
                            )
                            if kb >= qc * 8:  # diagonal block: causal mask
                                nc.vector.tensor_tensor(
                                    out=Pt[:, c0 : c0 + 512],
                                    in0=Pt[:, c0 : c0 + 512],
                                    in1=masks[(w0 - c0) // 128][:],
                                    op=ALU.mult,
                                )
                            pts.append(Pt)
                        for j in range(c0, 1024, 512):
                            for s in range(2):
                                nc.tensor.matmul(
                                    ymm[s][:, j : j + 512],
                                    vaug[:, s, kb * 128 : (kb + 1) * 128],
                                    pts[s][:, j : j + 512],
                                    start=(kb == 0),
                                    stop=(kb == klast[j // 512]),
                                )
                    for s in range(2):
                        den = rec_pool.tile([64, 1024], F32, tag="den")
                        nc.vector.tensor_copy(den[:], ymm[s][64:128, :])
                        rec = rec_pool.tile([64, 1024], F32, tag="rec")
                        nc.vector.reciprocal_approx_fast(rec[:], den[:])
                        nc.vector.tensor_tensor(
                            out=yT[64 * s : 64 * s + 64, p, q0 : q0 + 1024],
                            in0=ymm[s][0:64, :],
                            in1=rec[:],
                            op=ALU.mult,
                        )

        if debug:
            nc.sync.dma_start(qkvT_dbg[:], qkvT[:])
            nc.sync.dma_start(yT_dbg[:], yT[:])

        with (
            tc.tile_pool(name="ob", bufs=1) as ob_pool,
            tc.tile_pool(name="psC", bufs=8, space="PSUM") as psC,
        ):
            for m in range(8):
                pn = [
                    psC.tile([128, 512], F32, tag="pc", name=f"pc{m}_{n}")
                    for n in range(4)
                ]
                for k in range(4):
                    for n in range(4):
                        nc.tensor.matmul(
                            pn[n][:],
                            wps[:, k, m * 128 : (m + 1) * 128],
                            yT[:, k, n * 512 : (n + 1) * 512],
                            start=(k == 0),
                            stop=(k == 3),
                        )
                ob = ob_pool.tile([128, T], BF16, tag="ob")
                for n in range(4):
                    eng = nc.scalar if n < 2 else nc.vector
                    if n < 2:
                        nc.scalar.copy(ob[:, n * 512 : (n + 1) * 512], pn[n][:])
                    else:
                        nc.vector.tensor_copy(
                            ob[:, n * 512 : (n + 1) * 512], pn[n][:]
                        )
                nc.sync.dma_start(outT[m * 128 : (m + 1) * 128, :], ob[:])

    nc.compile()
    return nc


def _get_nc():
    if "nc" not in _CACHE:
        _CACHE["nc"] = _build()
    return _CACHE["nc"]


def _build_debug():
    return _build(debug=True)


def _prep_core_inputs(x, w_attn, b_attn, w_proj, b, g):
    cols = []
    for p in range(4):
        off = 512 * g + 128 * p
        cols += [
            w_attn[:, off : off + 128],
            w_attn[:, E + off : E + off + 128],
            w_attn[:, 2 * E + off : 2 * E + off + 128],
        ]
    wq = np.concatenate(cols, axis=1)  # [1024, 1536]
    wq = wq.reshape(8, 128, 12, 128).transpose(2, 1, 0, 3)  # [12, 128, 8, 128]
    wq = np.ascontiguousarray(wq, dtype=np.float32)
    bcols = []
    for p in range(4):
        off = 512 * g + 128 * p
        bcols += [
            b_attn[off : off + 128],
            b_attn[E + off : E + off + 128],
            b_attn[2 * E + off : 2 * E + off + 128],
        ]
    bq = np.stack(bcols, axis=1).astype(np.float32)  # [128, 12]
    wpr = np.concatenate(
        [w_proj[512 * g + 128 * p : 512 * g + 128 * p + 128, :] for p in range(4)],
        axis=0,
    )  # [512, 1024]
    wpr = np.ascontiguousarray(
        wpr.reshape(4, 128, 1024).transpose(1, 0, 2), dtype=np.float32
    )
    xT = np.ascontiguousarray(
        x[b].T.reshape(8, 128, T).transpose(1, 0, 2)
    )  # [128, 8, T]: [p, k, t] = x[b][t, 128k+p]
    return {
        "xin": xT.astype(ml_dtypes.bfloat16),
        "wqkv": wq.astype(ml_dtypes.bfloat16),
        "bqkv": np.ascontiguousarray(bq),
        "wp": wpr.astype(ml_dtypes.bfloat16),
    }


def kernel(x, w_attn, b_attn, w_proj, b_proj, _trace=False):
    from concourse.bass_utils import run_bass_kernel_spmd

    x = np.asarray(x, dtype=np.float32)
    w_attn = np.asarray(w_attn, dtype=np.float32)
    b_attn = np.asarray(b_attn, dtype=np.float32)
    w_proj = np.asarray(w_proj, dtype=np.float32)
    b_proj = np.asarray(b_proj, dtype=np.float32)

    nc = _get_nc()
    in_maps = [
        _prep_core_inputs(x, w_attn, b_attn, w_proj, core // 2, core % 2)
        for core in range(8)
    ]
    res = run_bass_kernel_spmd(nc, in_maps, core_ids=list(range(8)), trace=_trace)
    _CACHE["last_results"] = res
    out = np.empty((B, T, E), dtype=np.float32)
    for b in range(B):
        acc = res.results[2 * b]["outT"].astype(np.float32) + res.results[
            2 * b + 1
        ]["outT"].astype(np.float32)
        out[b] = acc.T + b_proj[None, :]
    return out


# revision 45
# speedup vs baseline: 1.4362x; 1.0001x over previous
"""Causal self-attention on 8 trn2 NeuronCores — v2.

Sharding: core = 2*b + g  (b in 0..3 batches, g in 0..1 head-groups of 8
heads). Host passes x^T per batch (so no on-chip transposes); per core:
  qkv^T = Wslice^T @ x^T   (feature-major), emitted interleaved with
  attention so the scalar engine's exp stream starts early.
  Attention in scores^T layout [k, q], both heads of a 128-partition
  group processed together (s=0 on PE rows 0:63, s=1 on rows 64:127).
  The AV lhsT is [V | ones] (128 cols), so the softmax denominator lands
  replicated on output partitions 64:128 — no cross-partition broadcast.
  Normalization: reciprocal_approx_fast + one multiply per (head, 1024 q).
  partial out^T = yT @ Wp_slice -> [1024, 2048], DMA'd from PSUM.
Host gathers: out[b] = (partial[2b] + partial[2b+1]).T + b_proj.
"""

import numpy as np
import ml_dtypes

B, T, E, H = 4, 2048, 1024, 16
HD = E // H  # 64

_CACHE = {}


def _build(debug=False):
    from contextlib import ExitStack

    import concourse.bass as bass
    import concourse.mybir as mybir
    import concourse.tile as tile
    from concourse import bacc

    F32 = mybir.dt.float32
    BF16 = mybir.dt.bfloat16
    AF = mybir.ActivationFunctionType
    ALU = mybir.AluOpType

    nc = bacc.Bacc("TRN2", target_bir_lowering=False)
    xin = nc.dram_tensor("xin", [128, 8, T], BF16, kind="ExternalInput")
    wqkv = nc.dram_tensor("wqkv", [12, 128, 8, 128], BF16, kind="ExternalInput")
    bqkv = nc.dram_tensor("bqkv", [128, 12], F32, kind="ExternalInput")
    wp = nc.dram_tensor("wp", [128, 4, 1024], BF16, kind="ExternalInput")
    outT = nc.dram_tensor("outT", [E, T], BF16, kind="ExternalOutput")
    if debug:
        qkvT_dbg = nc.dram_tensor(
            "qkvT_dbg", [128, 12, T], BF16, kind="ExternalOutput"
        )
        yT_dbg = nc.dram_tensor("yT_dbg", [128, 4, T], BF16, kind="ExternalOutput")

    with tile.TileContext(nc) as tc, ExitStack() as ctx:
        const = ctx.enter_context(tc.tile_pool(name="const", bufs=1))
        # stacked 64x64 identities at partitions 0 and 64 (v-transpose lhsT
        # sits at partition base 0 or 64)
        id2f = const.tile([128, 64], F32, tag="id2f")
        nc.gpsimd.memset(id2f[:], 0.0)
        for off in (0, 64):
            nc.gpsimd.affine_select(
                out=id2f[:],
                in_=id2f[:],
                compare_op=ALU.not_equal,
                fill=1.0,
                base=-off,
                pattern=[[-1, 64]],
                channel_multiplier=1,
            )
        id2 = const.tile([128, 64], BF16, tag="id2")
        nc.vector.tensor_copy(id2[:], id2f[:])
        from concourse.masks import make_identity

        identf = const.tile([128, 128], F32, tag="identf")
        make_identity(nc, identf[:])
        identr = const.tile([128, 128], BF16, tag="identr")
        nc.vector.tensor_copy(identr[:], identf[:])
        biasT = const.tile([128, 12], F32, tag="biasT")
        nc.sync.dma_start(biasT[:], bqkv[:])
        wps = const.tile([128, 4, 1024], BF16, tag="wps")
        ones64 = const.tile([128, 16, 64], BF16, tag="ones64")
        nc.gpsimd.memset(ones64[:], 1.0)
        # masks[j][p, col] = 1 iff col >= 128*j + p (causal keep in [k,q] layout)
        masks = []
        with tc.tile_pool(name="mtmp", bufs=2) as mtmp:
            for j in range(4):
                mjf = mtmp.tile([128, 512], F32, tag="maskf", name=f"maskf{j}")
                nc.gpsimd.memset(mjf[:], 1.0)
                nc.gpsimd.affine_select(
                    out=mjf[:],
                    in_=mjf[:],
                    compare_op=ALU.is_ge,
                    fill=0.0,
                    base=-128 * j,
                    pattern=[[1, 512]],
                    channel_multiplier=-1,
                )
                mj = const.tile([128, 512], BF16, tag=f"mask{j}", name=f"mask{j}")
                nc.vector.tensor_copy(mj[:], mjf[:])
                masks.append(mj)

        xT_pool = ctx.enter_context(tc.tile_pool(name="xT", bufs=1))
        xT = xT_pool.tile([128, 8, T], BF16, tag="xT")

        qkvT_pool = ctx.enter_context(tc.tile_pool(name="qkvT", bufs=1))
        qkvT = qkvT_pool.tile([128, 12, T], BF16, tag="qkvT")
        yT_pool = ctx.enter_context(tc.tile_pool(name="yT", bufs=1))
        yT = yT_pool.tile([128, 4, T], BF16, tag="yT")
        # zero-padded per-head K: scores then run with full 128-row
        # contraction (the zero rows null the other head's q), which keeps
        # the PE activity monitor warm through the attention phase.
        kpad_pool = ctx.enter_context(tc.tile_pool(name="kpad", bufs=1))
        kpads = []
        for i in range(2):
            kp = kpad_pool.tile([128, 2, T], BF16, tag=f"kpad{i}", name=f"kpad{i}")
            nc.gpsimd.memset(kp[:], 0.0)
            kpads.append(kp)
        # vaug[:, s, kb, 0:64] = V block (k rows on partitions), cols 64:128
        # stay at the initial memset value 1.0 (denominator ones)


        with (
            tc.tile_pool(name="wq", bufs=4) as wq_pool,
            tc.tile_pool(name="vaug", bufs=2) as vaug_pool,
            tc.tile_pool(name="Pp", bufs=3) as P_pool,
            tc.tile_pool(name="rec", bufs=2) as rec_pool,
            tc.tile_pool(name="psBs", bufs=2, space="PSUM") as psBs,
            tc.tile_pool(name="psBy", bufs=2, space="PSUM") as psBy,
        ):
            # P buffers are partially overwritten each use; the mask multiply
            # reads the full 512 chunk, so zero them once up front.
            for s in range(2):
                for i in range(3):
                    Pt0 = P_pool.tile([128, 1024], BF16, tag=f"P{s}", name=f"Pt0_{s}_{i}")
                    nc.gpsimd.memset(Pt0[:], 0.0)

            # DMA order matters: the first QKV matmul needs wqm[0] and
            # xT[k=0] only — put them first so the PE starts ~2us in, and
            # defer wps (phase C) to the end of the input stream.
            wqms = [
                wq_pool.tile([128, 8, 128], BF16, tag="wqm", name=f"wqm{m}")
                for m in range(12)
            ]
            nc.sync.dma_start(wqms[0][:], wqkv[0])
            nc.sync.dma_start(wqms[1][:], wqkv[1])
            nc.sync.dma_start(xT[:], xin[:])
            for m in range(2, 12):
                nc.sync.dma_start(wqms[m][:], wqkv[m])
            nc.sync.dma_start(wps[:], wp[:])

            for p in range(4):
                # ---- phase A chunk: qkv^T columns m = 3p..3p+2 ----
                for m in range(3 * p, 3 * p + 3):
                    for jj in range(2):
                        pq = psBs.tile([128, 1024], F32, tag="s", name=f"pq{m}_{jj}")
                        for j2 in range(2):
                            for k in range(8):
                                nc.tensor.matmul(
                                    pq[:, j2 * 512 : (j2 + 1) * 512],
                                    wqms[m][:, k, :],
                                    xT[
                                        :,
                                        k,
                                        (jj * 2 + j2) * 512 : (jj * 2 + j2 + 1) * 512,
                                    ],
                                    start=(k == 0),
                                    stop=(k == 7),
                                )
                        nc.vector.tensor_scalar_add(
                            qkvT[:, m, jj * 1024 : (jj + 1) * 1024],
                            pq[:],
                            biasT[:, m : m + 1],
                        )

                # ---- V transposes into vaug [k, V(64)|ones(64)] per kb.
                # One matmul per kb transposes BOTH heads (contraction over
                # all 128 dim-partitions, identity rhs), keeping N=128 and
                # the PE duty cycle high enough not to trip the HAM throttle.
                vaug = vaug_pool.tile([128, 2, 2048], BF16, tag="vaug")
                vaug_k = vaug[:, :, :].rearrange("p s (kb c) -> p s kb c", kb=16, c=128)
                for s in range(2):
                    nc.vector.tensor_copy(vaug_k[:, s, :, 64:128], ones64[:])
                for h in range(2):
                    pvt = psBs.tile([128, 1024], F32, tag="s")
                    for i in range(8):
                        kb = 8 * h + i
                        nc.tensor.matmul(
                            pvt[:, i * 128 : (i + 1) * 128],
                            qkvT[:, 3 * p + 2, kb * 128 : (kb + 1) * 128],
                            identr[:],
                            start=True,
                            stop=True,
                        )
                    pvt_k = pvt[:, :].rearrange("p (a c) -> p a c", a=8, c=128)
                    for s in range(2):
                        nc.vector.tensor_copy(
                            vaug_k[:, s, 8 * h : 8 * h + 8, 0:64],
                            pvt_k[:, :, 64 * s : 64 * s + 64],
                        )

                # ---- attention for heads (p, s=0), (p, s=1), s-fused ----
                kpad = kpads[p % 2]
                for s in range(2):
                    nc.vector.tensor_copy(
                        kpad[64 * s : 64 * s + 64, s, :],
                        qkvT[64 * s : 64 * s + 64, 3 * p + 1, :],
                    )
                qTf = qkvT[:, 3 * p, :]
                for qc in range(2):
                    q0 = qc * 1024
                    kmax = (qc + 1) * 8
                    klast = [
                        min(kmax, qc * 8 + (ci + 1) * 4) - 1 for ci in range(2)
                    ]
                    ymm = [
                        psBy.tile([128, 1024], F32, tag="ymm", name=f"ymm{p}_{qc}_{s}")
                        for s in range(2)
                    ]
                    for kb in range(kmax):
                        w0 = max(0, kb * 128 - q0)  # exact causal offset
                        c0 = (w0 // 512) * 512  # 512-aligned chunk start
                        sp = [
                            psBs.tile([128, 1024], F32, tag="s", name=f"sp{p}_{qc}_{kb}_{s}")
                            for s in range(2)
                        ]
                        for j in range(c0, 1024, 512):
                            for s in range(2):
                                nc.tensor.matmul(
                                    sp[s][:, j : j + 512],
                                    kpad[:, s, kb * 128 : (kb + 1) * 128],
                                    qTf[:, q0 + j : q0 + j + 512],
                                    start=True,
                                    stop=True,
                                )
                        pts = []
                        for s in range(2):
                            Pt = P_pool.tile(
                                [128, 1024], BF16, tag=f"P{s}", name=f"Pt{p}_{qc}_{kb}_{s}"
                            )
                            nc.scalar.activation(
                                Pt[:, w0:1024], sp[s][:, w0:1024], AF.Exp, scale=0.125
                            )
                            if kb >= qc * 8:  # diagonal block: causal mask
                                nc.vector.tensor_tensor(
                                    out=Pt[:, c0 : c0 + 512],
                                    in0=Pt[:, c0 : c0 + 512],
                                    in1=masks[(w0 - c0) // 128][:],
                                    op=ALU.mult,
                                )
                            pts.append(Pt)
                        for j in range(c0, 1024, 512):
                            for s in range(2):
                                nc.tensor.matmul(
                                    ymm[s][:, j : j + 512],
                                    vaug[:, s, kb * 128 : (kb + 1) * 128],
                                    pts[s][:, j : j + 512],
                                    start=(kb == 0),
                                    stop=(kb == klast[j // 512]),
                                )
                    for s in range(2):
                        den = rec_pool.tile([64, 1024], F32, tag="den")
                        nc.vector.tensor_copy(den[:], ymm[s][64:128, :])
                        rec = rec_pool.tile([64, 1024], F32, tag="rec")
                        nc.vector.reciprocal_approx_fast(rec[:], den[:])
                        nc.vector.tensor_tensor(
                            out=yT[64 * s : 64 * s + 64, p, q0 : q0 + 1024],
                            in0=ymm[s][0:64, :],
                            in1=rec[:],
                            op=ALU.mult,
                        )

        if debug:
            nc.sync.dma_start(qkvT_dbg[:], qkvT[:])
            nc.sync.dma_start(yT_dbg[:], yT[:])

        with (
            tc.tile_pool(name="ob", bufs=2) as ob_pool,
            tc.tile_pool(name="psC", bufs=8, space="PSUM") as psC,
        ):
            pns = {}
            for m in range(2):
                pns[m] = [
                    psC.tile([128, 512], F32, tag="pc", name=f"pc{m}_{n}")
                    for n in range(4)
                ]
                for k in range(3):
                    for n in range(4):
                        nc.tensor.matmul(
                            pns[m][n][:],
                            wps[:, k, m * 128 : (m + 1) * 128],
                            yT[:, k, n * 512 : (n + 1) * 512],
                            start=(k == 0),
                            stop=False,
                        )
            for m in range(8):
                if m < 2:
                    pn = pns[m]
                    for n in range(4):
                        nc.tensor.matmul(
                            pn[n][:],
                            wps[:, 3, m * 128 : (m + 1) * 128],
                            yT[:, 3, n * 512 : (n + 1) * 512],
                            start=False,
                            stop=True,
                        )
                else:
                    pn = [
                        psC.tile([128, 512], F32, tag="pc", name=f"pc{m}_{n}")
                        for n in range(4)
                    ]
                    for k in range(4):
                        for n in range(4):
                            nc.tensor.matmul(
                                pn[n][:],
                                wps[:, k, m * 128 : (m + 1) * 128],
                                yT[:, k, n * 512 : (n + 1) * 512],
                                start=(k == 0),
                                stop=(k == 3),
                            )
                ob = ob_pool.tile([128, T], BF16, tag="ob")
                for n in range(4):
                    if n < 2:
                        nc.scalar.copy(ob[:, n * 512 : (n + 1) * 512], pn[n][:])
                    else:
                        nc.vector.tensor_copy(
                            ob[:, n * 512 : (n + 1) * 512], pn[n][:]
                        )
                nc.sync.dma_start(outT[m * 128 : (m + 1) * 128, :], ob[:])

    nc.compile()
    return nc


def _get_nc():
    if "nc" not in _CACHE:
        _CACHE["nc"] = _build()
    return _CACHE["nc"]


def _build_debug():
    return _build(debug=True)


def _prep_core_inputs(x, w_attn, b_attn, w_proj, b, g):
    cols = []
    for p in range(4):
        off = 512 * g + 128 * p
        cols += [
            w_attn[:, off : off + 128],
            w_attn[:, E + off : E + off + 128],
            w_attn[:, 2 * E + off : 2 * E + off + 128],
        ]
    wq = np.concatenate(cols, axis=1)  # [1024, 1536]
    wq = wq.reshape(8, 128, 12, 128).transpose(2, 1, 0, 3)  # [12, 128, 8, 128]
    wq = np.ascontiguousarray(wq, dtype=np.float32)
    bcols = []
    for p in range(4):
        off = 512 * g + 128 * p
        bcols += [
            b_attn[off : off + 128],
            b_attn[E + off : E + off + 128],
            b_attn[2 * E + off : 2 * E + off + 128],
        ]
    bq = np.stack(bcols, axis=1).astype(np.float32)  # [128, 12]
    wpr = np.concatenate(
        [w_proj[512 * g + 128 * p : 512 * g + 128 * p + 128, :] for p in range(4)],
        axis=0,
    )  # [512, 1024]
    wpr = np.ascontiguousarray(
        wpr.reshape(4, 128, 1024).transpose(1, 0, 2), dtype=np.float32
    )
    xT = np.ascontiguousarray(
        x[b].T.reshape(8, 128, T).transpose(1, 0, 2)
    )  # [128, 8, T]: [p, k, t] = x[b][t, 128k+p]
    return {
        "xin": xT.astype(ml_dtypes.bfloat16),
        "wqkv": wq.astype(ml_dtypes.bfloat16),
        "bqkv": np.ascontiguousarray(bq),
        "wp": wpr.astype(ml_dtypes.bfloat16),
    }


def kernel(x, w_attn, b_attn, w_proj, b_proj, _trace=False):
    from concourse.bass_utils import run_bass_kernel_spmd

    x = np.asarray(x, dtype=np.float32)
    w_attn = np.asarray(w_attn, dtype=np.float32)
    b_attn = np.asarray(b_attn, dtype=np.float32)
    w_proj = np.asarray(w_proj, dtype=np.float32)
    b_proj = np.asarray(b_proj, dtype=np.float32)

    nc = _get_nc()
    in_maps = [
        _prep_core_inputs(x, w_attn, b_attn, w_proj, core // 2, core % 2)
        for core in range(8)
    ]
    res = run_bass_kernel_spmd(nc, in_maps, core_ids=list(range(8)), trace=_trace)
    _CACHE["last_results"] = res
    out = np.empty((B, T, E), dtype=np.float32)
    for b in range(B):
        acc = res.results[2 * b]["outT"].astype(np.float32) + res.results[
            2 * b + 1
        ]["outT"].astype(np.float32)
        out[b] = acc.T + b_proj[None, :]
    return out


# revision 46
# speedup vs baseline: 1.4643x; 1.0195x over previous
"""Causal self-attention on 8 trn2 NeuronCores — v2.

Sharding: core = 2*b + g  (b in 0..3 batches, g in 0..1 head-groups of 8
heads). Host passes x^T per batch (so no on-chip transposes); per core:
  qkv^T = Wslice^T @ x^T   (feature-major), emitted interleaved with
  attention so the scalar engine's exp stream starts early.
  Attention in scores^T layout [k, q], both heads of a 128-partition
  group processed together (s=0 on PE rows 0:63, s=1 on rows 64:127).
  The AV lhsT is [V | ones] (128 cols), so the softmax denominator lands
  replicated on output partitions 64:128 — no cross-partition broadcast.
  Normalization: reciprocal_approx_fast + one multiply per (head, 1024 q).
  partial out^T = yT @ Wp_slice -> [1024, 2048], DMA'd from PSUM.
Host gathers: out[b] = (partial[2b] + partial[2b+1]).T + b_proj.
"""

import numpy as np
import ml_dtypes

B, T, E, H = 4, 2048, 1024, 16
HD = E // H  # 64

_CACHE = {}


def _build(debug=False):
    from contextlib import ExitStack

    import concourse.bass as bass
    import concourse.mybir as mybir
    import concourse.tile as tile
    from concourse import bacc

    F32 = mybir.dt.float32
    BF16 = mybir.dt.bfloat16
    AF = mybir.ActivationFunctionType
    ALU = mybir.AluOpType

    nc = bacc.Bacc("TRN2", target_bir_lowering=False)
    xin = nc.dram_tensor("xin", [128, 8, T], BF16, kind="ExternalInput")
    wqkv = nc.dram_tensor("wqkv", [12, 128, 8, 128], BF16, kind="ExternalInput")
    bqkv = nc.dram_tensor("bqkv", [128, 12], F32, kind="ExternalInput")
    wp = nc.dram_tensor("wp", [128, 4, 1024], BF16, kind="ExternalInput")
    outT = nc.dram_tensor("outT", [E, T], BF16, kind="ExternalOutput")
    if debug:
        qkvT_dbg = nc.dram_tensor(
            "qkvT_dbg", [128, 12, T], BF16, kind="ExternalOutput"
        )
        yT_dbg = nc.dram_tensor("yT_dbg", [128, 4, T], BF16, kind="ExternalOutput")

    with tile.TileContext(nc) as tc, ExitStack() as ctx:
        const = ctx.enter_context(tc.tile_pool(name="const", bufs=1))
        # stacked 64x64 identities at partitions 0 and 64 (v-transpose lhsT
        # sits at partition base 0 or 64)
        id2f = const.tile([128, 64], F32, tag="id2f")
        nc.gpsimd.memset(id2f[:], 0.0)
        for off in (0, 64):
            nc.gpsimd.affine_select(
                out=id2f[:],
                in_=id2f[:],
                compare_op=ALU.not_equal,
                fill=1.0,
                base=-off,
                pattern=[[-1, 64]],
                channel_multiplier=1,
            )
        id2 = const.tile([128, 64], BF16, tag="id2")
        nc.vector.tensor_copy(id2[:], id2f[:])
        from concourse.masks import make_identity

        identf = const.tile([128, 128], F32, tag="identf")
        make_identity(nc, identf[:])
        identr = const.tile([128, 128], BF16, tag="identr")
        nc.vector.tensor_copy(identr[:], identf[:])
        biasT = const.tile([128, 12], F32, tag="biasT")
        nc.sync.dma_start(biasT[:], bqkv[:])
        wps = const.tile([128, 4, 1024], BF16, tag="wps")
        ones64 = const.tile([128, 16, 64], BF16, tag="ones64")
        nc.gpsimd.memset(ones64[:], 1.0)
        # masks[j][p, col] = 1 iff col >= 128*j + p (causal keep in [k,q] layout)
        masks = []
        with tc.tile_pool(name="mtmp", bufs=2) as mtmp:
            for j in range(4):
                mjf = mtmp.tile([128, 512], F32, tag="maskf", name=f"maskf{j}")
                nc.gpsimd.memset(mjf[:], 1.0)
                nc.gpsimd.affine_select(
                    out=mjf[:],
                    in_=mjf[:],
                    compare_op=ALU.is_ge,
                    fill=0.0,
                    base=-128 * j,
                    pattern=[[1, 512]],
                    channel_multiplier=-1,
                )
                mj = const.tile([128, 512], BF16, tag=f"mask{j}", name=f"mask{j}")
                nc.vector.tensor_copy(mj[:], mjf[:])
                masks.append(mj)

        xT_pool = ctx.enter_context(tc.tile_pool(name="xT", bufs=1))
        xT = xT_pool.tile([128, 8, T], BF16, tag="xT")

        qkvT_pool = ctx.enter_context(tc.tile_pool(name="qkvT", bufs=1))
        qkvT = qkvT_pool.tile([128, 12, T], BF16, tag="qkvT")
        yT_pool = ctx.enter_context(tc.tile_pool(name="yT", bufs=1))
        yT = yT_pool.tile([128, 4, T], BF16, tag="yT")
        # zero-padded per-head K: scores then run with full 128-row
        # contraction (the zero rows null the other head's q), which keeps
        # the PE activity monitor warm through the attention phase.
        kpad_pool = ctx.enter_context(tc.tile_pool(name="kpad", bufs=1))
        kpads = []
        for i in range(2):
            kp = kpad_pool.tile([128, 2, T], BF16, tag=f"kpad{i}", name=f"kpad{i}")
            nc.gpsimd.memset(kp[:], 0.0)
            kpads.append(kp)
        # vaug[:, s, kb, 0:64] = V block (k rows on partitions), cols 64:128
        # stay at the initial memset value 1.0 (denominator ones)


        with (
            tc.tile_pool(name="wq", bufs=4) as wq_pool,
            tc.tile_pool(name="vaug", bufs=2) as vaug_pool,
            tc.tile_pool(name="Pp", bufs=3) as P_pool,
            tc.tile_pool(name="rec", bufs=2) as rec_pool,
            tc.tile_pool(name="psBs", bufs=2, space="PSUM") as psBs,
            tc.tile_pool(name="psBy", bufs=2, space="PSUM") as psBy,
        ):
            # P buffers are partially overwritten each use; the mask multiply
            # reads the full 512 chunk, so zero them once up front.
            for s in range(2):
                for i in range(3):
                    Pt0 = P_pool.tile([128, 1024], BF16, tag=f"P{s}", name=f"Pt0_{s}_{i}")
                    nc.gpsimd.memset(Pt0[:], 0.0)

            # DMA order matters: the first QKV matmul needs wqm[0] and
            # xT[k=0] only — put them first so the PE starts ~2us in, and
            # defer wps (phase C) to the end of the input stream.
            wqms = [
                wq_pool.tile([128, 8, 128], BF16, tag="wqm", name=f"wqm{m}")
                for m in range(12)
            ]
            nc.sync.dma_start(wqms[0][:], wqkv[0])
            nc.sync.dma_start(wqms[1][:], wqkv[1])
            for k in range(8):
                nc.sync.dma_start(xT[:, k, :], xin[:, k, :])
            for m in range(2, 12):
                nc.sync.dma_start(wqms[m][:], wqkv[m])
            nc.sync.dma_start(wps[:], wp[:])

            for p in range(4):
                # ---- phase A chunk: qkv^T columns m = 3p..3p+2 ----
                for m in range(3 * p, 3 * p + 3):
                    for jj in range(2):
                        pq = psBs.tile([128, 1024], F32, tag="s", name=f"pq{m}_{jj}")
                        for j2 in range(2):
                            for k in range(8):
                                nc.tensor.matmul(
                                    pq[:, j2 * 512 : (j2 + 1) * 512],
                                    wqms[m][:, k, :],
                                    xT[
                                        :,
                                        k,
                                        (jj * 2 + j2) * 512 : (jj * 2 + j2 + 1) * 512,
                                    ],
                                    start=(k == 0),
                                    stop=(k == 7),
                                )
                        nc.vector.tensor_scalar_add(
                            qkvT[:, m, jj * 1024 : (jj + 1) * 1024],
                            pq[:],
                            biasT[:, m : m + 1],
                        )

                # ---- V transposes into vaug [k, V(64)|ones(64)] per kb.
                # One matmul per kb transposes BOTH heads (contraction over
                # all 128 dim-partitions, identity rhs), keeping N=128 and
                # the PE duty cycle high enough not to trip the HAM throttle.
                vaug = vaug_pool.tile([128, 2, 2048], BF16, tag="vaug")
                vaug_k = vaug[:, :, :].rearrange("p s (kb c) -> p s kb c", kb=16, c=128)
                for s in range(2):
                    nc.vector.tensor_copy(vaug_k[:, s, :, 64:128], ones64[:])
                for h in range(2):
                    pvt = psBs.tile([128, 1024], F32, tag="s")
                    for i in range(8):
                        kb = 8 * h + i
                        nc.tensor.matmul(
                            pvt[:, i * 128 : (i + 1) * 128],
                            qkvT[:, 3 * p + 2, kb * 128 : (kb + 1) * 128],
                            identr[:],
                            start=True,
                            stop=True,
                        )
                    pvt_k = pvt[:, :].rearrange("p (a c) -> p a c", a=8, c=128)
                    for s in range(2):
                        nc.vector.tensor_copy(
                            vaug_k[:, s, 8 * h : 8 * h + 8, 0:64],
                            pvt_k[:, :, 64 * s : 64 * s + 64],
                        )

                # ---- attention for heads (p, s=0), (p, s=1), s-fused ----
                kpad = kpads[p % 2]
                for s in range(2):
                    nc.vector.tensor_copy(
                        kpad[64 * s : 64 * s + 64, s, :],
                        qkvT[64 * s : 64 * s + 64, 3 * p + 1, :],
                    )
                qTf = qkvT[:, 3 * p, :]
                for qc in range(2):
                    q0 = qc * 1024
                    kmax = (qc + 1) * 8
                    klast = [
                        min(kmax, qc * 8 + (ci + 1) * 4) - 1 for ci in range(2)
                    ]
                    ymm = [
                        psBy.tile([128, 1024], F32, tag="ymm", name=f"ymm{p}_{qc}_{s}")
                        for s in range(2)
                    ]
                    for kb in range(kmax):
                        w0 = max(0, kb * 128 - q0)  # exact causal offset
                        c0 = (w0 // 512) * 512  # 512-aligned chunk start
                        sp = [
                            psBs.tile([128, 1024], F32, tag="s", name=f"sp{p}_{qc}_{kb}_{s}")
                            for s in range(2)
                        ]
                        for j in range(c0, 1024, 512):
                            for s in range(2):
                                nc.tensor.matmul(
                                    sp[s][:, j : j + 512],
                                    kpad[:, s, kb * 128 : (kb + 1) * 128],
                                    qTf[:, q0 + j : q0 + j + 512],
                                    start=True,
                                    stop=True,
                                )
                        pts = []
                        for s in range(2):
                            Pt = P_pool.tile(
                                [128, 1024], BF16, tag=f"P{s}", name=f"Pt{p}_{qc}_{kb}_{s}"
                            )
                            nc.scalar.activation(
                                Pt[:, w0:1024], sp[s][:, w0:1024], AF.Exp, scale=0.125
                            )
                            if kb >= qc * 8:  # diagonal block: causal mask
                                nc.vector.tensor_tensor(
                                    out=Pt[:, c0 : c0 + 512],
                                    in0=Pt[:, c0 : c0 + 512],
                                    in1=masks[(w0 - c0) // 128][:],
                                    op=ALU.mult,
                                )
                            pts.append(Pt)
                        for j in range(c0, 1024, 512):
                            for s in range(2):
                                nc.tensor.matmul(
                                    ymm[s][:, j : j + 512],
                                    vaug[:, s, kb * 128 : (kb + 1) * 128],
                                    pts[s][:, j : j + 512],
                                    start=(kb == 0),
                                    stop=(kb == klast[j // 512]),
                                )
                    for s in range(2):
                        den = rec_pool.tile([64, 1024], F32, tag="den")
                        nc.vector.tensor_copy(den[:], ymm[s][64:128, :])
                        rec = rec_pool.tile([64, 1024], F32, tag="rec")
                        nc.vector.reciprocal_approx_fast(rec[:], den[:])
                        nc.vector.tensor_tensor(
                            out=yT[64 * s : 64 * s + 64, p, q0 : q0 + 1024],
                            in0=ymm[s][0:64, :],
                            in1=rec[:],
                            op=ALU.mult,
                        )

            # ---- phase C: output projection (same pool block: avoids the
            # PSUM pool-close barrier that cost ~7us before C started) ----
            with tc.tile_pool(name="ob", bufs=2) as ob_pool:
                for m in range(8):
                    pn = [
                        psBs.tile([128, 1024], F32, tag="s", name=f"pc{m}_{h}")
                        for h in range(2)
                    ]
                    for k in range(4):
                        for n in range(4):
                            nc.tensor.matmul(
                                pn[n // 2][:, (n % 2) * 512 : (n % 2) * 512 + 512],
                                wps[:, k, m * 128 : (m + 1) * 128],
                                yT[:, k, n * 512 : (n + 1) * 512],
                                start=(k == 0),
                                stop=(k == 3),
                            )
                    ob = ob_pool.tile([128, T], BF16, tag="ob")
                    for h in range(2):
                        if h == 0:
                            nc.scalar.copy(ob[:, 0:1024], pn[0][:])
                        else:
                            nc.vector.tensor_copy(ob[:, 1024:2048], pn[1][:])
                    nc.sync.dma_start(outT[m * 128 : (m + 1) * 128, :], ob[:])

        if debug:
            nc.sync.dma_start(qkvT_dbg[:], qkvT[:])
            nc.sync.dma_start(yT_dbg[:], yT[:])

    nc.compile()
    return nc


def _get_nc():
    if "nc" not in _CACHE:
        _CACHE["nc"] = _build()
    return _CACHE["nc"]


def _build_debug():
    return _build(debug=True)


def _prep_core_inputs(x, w_attn, b_attn, w_proj, b, g):
    cols = []
    for p in range(4):
        off = 512 * g + 128 * p
        cols += [
            w_attn[:, off : off + 128],
            w_attn[:, E + off : E + off + 128],
            w_attn[:, 2 * E + off : 2 * E + off + 128],
        ]
    wq = np.concatenate(cols, axis=1)  # [1024, 1536]
    wq = wq.reshape(8, 128, 12, 128).transpose(2, 1, 0, 3)  # [12, 128, 8, 128]
    wq = np.ascontiguousarray(wq, dtype=np.float32)
    bcols = []
    for p in range(4):
        off = 512 * g + 128 * p
        bcols += [
            b_attn[off : off + 128],
            b_attn[E + off : E + off + 128],
            b_attn[2 * E + off : 2 * E + off + 128],
        ]
    bq = np.stack(bcols, axis=1).astype(np.float32)  # [128, 12]
    wpr = np.concatenate(
        [w_proj[512 * g + 128 * p : 512 * g + 128 * p + 128, :] for p in range(4)],
        axis=0,
    )  # [512, 1024]
    wpr = np.ascontiguousarray(
        wpr.reshape(4, 128, 1024).transpose(1, 0, 2), dtype=np.float32
    )
    xT = np.ascontiguousarray(
        x[b].T.reshape(8, 128, T).transpose(1, 0, 2)
    )  # [128, 8, T]: [p, k, t] = x[b][t, 128k+p]
    return {
        "xin": xT.astype(ml_dtypes.bfloat16),
        "wqkv": wq.astype(ml_dtypes.bfloat16),
        "bqkv": np.ascontiguousarray(bq),
        "wp": wpr.astype(ml_dtypes.bfloat16),
    }


def kernel(x, w_attn, b_attn, w_proj, b_proj, _trace=False):
    from concourse.bass_utils import run_bass_kernel_spmd

    x = np.asarray(x, dtype=np.float32)
    w_attn = np.asarray(w_attn, dtype=np.float32)
    b_attn = np.asarray(b_attn, dtype=np.float32)
    w_proj = np.asarray(w_proj, dtype=np.float32)
    b_proj = np.asarray(b_proj, dtype=np.float32)

    nc = _get_nc()
    in_maps = [
        _prep_core_inputs(x, w_attn, b_attn, w_proj, core // 2, core % 2)
        for core in range(8)
    ]
    res = run_bass_kernel_spmd(nc, in_maps, core_ids=list(range(8)), trace=_trace)
    _CACHE["last_results"] = res
    out = np.empty((B, T, E), dtype=np.float32)
    for b in range(B):
        acc = res.results[2 * b]["outT"].astype(np.float32) + res.results[
            2 * b + 1
        ]["outT"].astype(np.float32)
        out[b] = acc.T + b_proj[None, :]
    return out
